# revision 73
# baseline (speedup 1.0000x reference)
"""Trainium2 Bass kernel for the AbstractQCP residual operator F @ W.

Math (reference):
    v = y - s; mask = (v >= 0)
    dx = wx; dy = mask*wy; dt = wt        (W = [wx; wy; wt], (n+m+1, K))
    o1 = P@wx + A.T@dy + q wt             (n, K)
    o2 = b wt - A@wx                      (m, K)
    o3 = (x.T P x) wt - (q + 2 P x)@wx - b@dy
    F  = [o1; o2 + (1-mask)*wy; o3]

Design (per core i of 8, pure SPMD, host gathers):
  core i owns o1 rows [512i,512(i+1)) and o2 rows [1024i,1024(i+1)).
  Host precomputes: mask, row-compacted A.T@dy operands (only rows with
  mask=1 contribute), Px = P@x, xTPx, cf = -(q+2Px).
  All big operands fp8 E3M4 scaled by a single power-of-two S (=64):
    G1P: lhsT = S*P[:,cols_i] (32 ktiles; P symmetric), rhs = wx8 = wx
    G1A: lhsT = S*A[maskrows, cols_i] (compacted ktiles), rhs = dy8
    q x) wt: contraction-1 bf16 matmul, lhsT = S*q_i, rhs = wt row
    --> all accumulate in ONE psum set (identical scale); o1 = ps1/S.
    G2:  lhsT = -S*A[rows_i,:].T (32 ktiles), rhs = wx8 (shared tiles!);
         b wt via contraction-1 bf16 matmul lhsT = S*b_i.
         o2 = ps2/S + (1-mask)*wy.  Optional e4m3+DoubleRow mode.
  o3 partial per core: cf@wx_i + (-b_i)@(mask*wy_i); host adds xTPx*wt.
  PSUM: 2 banks o1 + 4 banks G2 (2 x 256-wide accumulators per bank,
  bank-shared start/stop flags) + 1 bank o3.
  DMA: ~11.6 MB/core balanced over the 3 trigger queues (sync/scalar/
  gpsimd), ~0.5-1 MB per transfer.

Streamed operands staged in DRAM K-tile-transposed: (128, ktiles*free)
with element (p, k*free+c) = orig(k*128+p, c).
"""

import numpy as np
import ml_dtypes
from contextlib import ExitStack

BF = ml_dtypes.bfloat16
E3 = ml_dtypes.float8_e3m4
E4 = ml_dtypes.float8_e4m3

N, M, KP = 4096, 8192, 256
NC = 8
NS, MS = N // NC, M // NC          # 512, 1024
KTP = 32                           # P k-tiles
KT2 = 32                           # G2 k-tiles (full n contraction)

G2_MODE = "drsw"                   # 'e3' | 'drsw' (e4m3 + DoubleRowSwInterleave)

_NC_CACHE = {}


def _kt(a, ktiles, free):
    """(ktiles*128, free) row-major -> (128, ktiles*free) K-tile-transposed."""
    return np.ascontiguousarray(
        a.reshape(ktiles, 128, free).transpose(1, 0, 2).reshape(128, ktiles * free))


def _build_nc(kta, g2_dr, c_inv):
    from concourse import bacc, tile, mybir
    from concourse.alu_op_type import AluOpType as op

    dtb = mybir.dt.bfloat16
    dtf = mybir.dt.float32
    dt8 = mybir.dt.float8e3
    dt8c = mybir.dt.float8e4 if g2_dr else mybir.dt.float8e3
    pm = mybir.MatmulPerfMode.DoubleRowSwInterleave if g2_dr else None

    nc = bacc.Bacc("TRN2", target_bir_lowering=False, debug=False)

    def din(name, shape, dt):
        return nc.dram_tensor(name, list(shape), dt, kind="ExternalInput").ap()

    pt8 = din("pt8", (128, KTP * NS), dt8)    # S*P[:,cols] K-tiled
    at8 = din("at8", (128, kta * NS), dt8)    # compacted S*A rows, K-tiled
    dy8 = din("dy8", (128, kta * KP), dt8)    # compacted wy, K-tiled
    if g2_dr:
        # SW-interleaved pairs: (pair, mtile, [2*(127-m)+i]) per partition
        ct8 = din("ct8", (128, KT2 // 2, 8 * 2 * 128), dt8c)
    else:
        ct8 = din("ct8", (128, KT2, MS), dt8c)  # -S*A[rows].T K-tiled
    wx8 = din("wx8", (128, KT2, KP), dt8c)    # wx K-tiled (G1P + G2 rhs)
    # row smalls: [S*q_i | wt | S*b_i] in one row tensor
    rsd = din("rs", (1, NS + KP + MS), dtb)
    # col smalls: [cf(4) | -b(8) | y(8) | s(8)] as fp32 columns
    csd = din("cs", (128, 28), dtf)
    wod = din("wosb", (128, 12 * KP), dtb)    # [own wy (8) | own wx (4)] K-tiled
    out1 = nc.dram_tensor("out1", [128, 4 * KP], dtb, kind="ExternalOutput").ap()
    out2 = nc.dram_tensor("out2", [128, 8 * KP], dtb, kind="ExternalOutput").ap()
    out3 = nc.dram_tensor("out3", [1, KP], dtf, kind="ExternalOutput").ap()

    NSTEP = KTP + kta
    PG = [0, 8, 16, 24, 32]                   # pt8 groups (alternate sync/scalar)
    AG = sorted(set(min(b, kta) for b in [0, 8, 16, 24, kta]))  # at8 (alt)
    # ct8 groups (gpsimd): in pair units for drsw, ktile units for e3
    CG = [0, 4, 8, 12, 16] if g2_dr else [0, 8, 16, 24, 32]
    WXG = [0, 8, 32]                          # wx8 chunks (scalar)

    def g_of(bounds):
        m = {}
        for g in range(len(bounds) - 1):
            for k in range(bounds[g], bounds[g + 1]):
                m[k] = g
        return m

    pg_of, ag_of, cg_of = g_of(PG), g_of(AG), g_of(CG)

    nticks = KT2 // 2 if g2_dr else KT2
    first_tick, last_tick = 14, NSTEP - 10
    tick_step = [first_tick + round(t * (last_tick - first_tick) / (nticks - 1))
                 for t in range(nticks)]
    t2s = {}
    for t, s_ in enumerate(tick_step):
        t2s.setdefault(s_, []).append(t)
    ct_load_step = {}
    for g in range(len(CG) - 1):
        ct_load_step.setdefault(max(3, tick_step[min(CG[g], nticks - 1)] - 6),
                                []).append(g)

    with tile.TileContext(nc) as tc, ExitStack() as ctx:
        dpool = ctx.enter_context(tc.tile_pool(name="d", bufs=1))
        ppool = ctx.enter_context(tc.tile_pool(name="p", bufs=4))
        apool = ctx.enter_context(tc.tile_pool(name="a", bufs=4))
        ypool = ctx.enter_context(tc.tile_pool(name="y", bufs=2))
        cpool = ctx.enter_context(tc.tile_pool(name="c", bufs=2))
        opool = ctx.enter_context(tc.tile_pool(name="o", bufs=1))
        pspool = ctx.enter_context(tc.tile_pool(name="ps", bufs=8, space="PSUM"))

        ps1 = [pspool.tile((128, 2 * KP), dtf, tag="ps", name=f"ps1{i}") for i in range(2)]
        ps2 = [pspool.tile((128, 2 * KP), dtf, tag="ps", name=f"ps2{i}") for i in range(4)]

        def pslot(tiles, t):
            return tiles[t // 2][:, (t % 2) * KP:(t % 2 + 1) * KP]

        ptg, atg, dyg, ctg = {}, {}, {}, {}

        def load_p(g):
            k0, k1 = PG[g], PG[g + 1]
            t = ppool.tile((128, (k1 - k0) * NS), dt8, tag="pt", name=f"ptg{g}",
                           padded_shape=(128, 10 * NS))
            eng = nc.sync if g % 2 == 0 else nc.scalar
            eng.dma_start(t, pt8[:, k0 * NS:k1 * NS])
            ptg[g] = t

        def load_a(g):
            k0, k1 = AG[g], AG[g + 1]
            t = apool.tile((128, (k1 - k0) * NS), dt8, tag="at",
                           name=f"atg{g}", padded_shape=(128, 9 * NS))
            eng = nc.sync if g % 2 == 0 else nc.scalar
            eng.dma_start(t, at8[:, k0 * NS:k1 * NS])
            atg[g] = t

        def load_y():
            t = ypool.tile((128, kta * KP), dt8, tag="dy", name="dyg")
            nc.scalar.dma_start(t, dy8)
            dyg[0] = t

        def load_c(g):
            j0, j1 = CG[g], CG[g + 1]
            gmax = max(b - a for a, b in zip(CG[:-1], CG[1:]))
            if g2_dr:
                t = cpool.tile((128, gmax, 8 * 2 * 128), dt8c, tag="ct", name=f"ctg{g}")
            else:
                t = cpool.tile((128, gmax, MS), dt8c, tag="ct", name=f"ctg{g}")
            nc.gpsimd.dma_start(t[:, 0:j1 - j0, :], ct8[:, j0:j1, :])
            ctg[g] = t

        # wx8 resident: two fat-row transfers on gpsimd into SEPARATE tiles
        # (separate tile objects -> per-half dependency granularity)
        H2 = KT2 // 2
        wxta = dpool.tile((128, H2, KP), dt8c, tag="wxta", name="wxta")
        wxtb = dpool.tile((128, KT2 - H2, KP), dt8c, tag="wxtb", name="wxtb")

        def wx8_rhs(j):
            return wxta[:, j, :] if j < H2 else wxtb[:, j - H2, :]

        def wx8_rhs_pair(p_):
            j = 2 * p_
            return (wxta[:, j:j + 2, 0:KP] if j < H2
                    else wxtb[:, j - H2:j - H2 + 2, 0:KP])

        sm = {}

        def emit_first_smalls():
            rs = dpool.tile((1, NS + KP + MS), dtb, tag="rs", name="rs")
            nc.scalar.dma_start(rs, rsd)
            sm["rs"] = rs

        def emit_smalls():
            cs = dpool.tile((128, 28), dtf, tag="cs", name="cs")
            nc.scalar.dma_start(cs, csd)
            sm["cs"] = cs

        def emit_masks():
            cs = sm["cs"]
            vo = dpool.tile((128, 8), dtf, tag="vo", name="vo")
            nc.vector.tensor_sub(vo, cs[:, 12:20], cs[:, 20:28])
            masko = dpool.tile((128, 8), dtf, tag="masko", name="masko")
            nc.vector.tensor_scalar(masko, vo, 0.0, None, op.is_ge)
            umo = dpool.tile((128, 8), dtf, tag="umo", name="umo")
            nc.vector.tensor_scalar(umo, masko, -1.0, 1.0, op.mult, op.add)
            sm["umo"] = umo
            cnb = dpool.tile((128, 12), dtb, tag="cnb", name="cnb")
            nc.vector.tensor_copy(cnb, cs[:, 0:12])
            sm["cnb"] = cnb

        def emit_wom():
            wom = dpool.tile((128, 8 * KP), dtb, tag="wom", name="wom")
            wmt = dpool.tile((128, 8 * KP), dtb, tag="wmt", name="wmt")
            for t_i in range(8):
                sl = slice(t_i * KP, (t_i + 1) * KP)
                nc.vector.tensor_scalar_mul(wom[:, sl], sm["wos"][:, sl],
                                            sm["umo"][:, t_i:t_i + 1])
            for t_i in range(8):
                sl = slice(t_i * KP, (t_i + 1) * KP)
                nc.vector.tensor_sub(wmt[:, sl], sm["wos"][:, sl], wom[:, sl])
            sm["wom"] = wom
            sm["wmt"] = wmt

        from bass_rust import ActivationFunctionType as AFT

        ob1 = opool.tile((128, 4 * KP), dtb, tag="ob1", name="ob1")
        ob2 = opool.tile((128, 8 * KP), dtb, tag="ob2", name="ob2")

        # front-loaded triggers: smalls + wx8 c0/c1 on scalar, pt g0/g1 on sync
        emit_first_smalls()
        nc.gpsimd.dma_start(wxta, wx8[:, 0:H2, :])
        nc.gpsimd.dma_start(wxtb, wx8[:, H2:KT2, :])
        load_p(0)
        load_p(1)

        done_ticks = 0
        for k in range(NSTEP):
            is_p = k < KTP
            kk = k if is_p else k - KTP

            # --- JIT stream prefetch ---
            if is_p:
                g = pg_of[kk]
                if kk == PG[g] and g + 2 <= len(PG) - 2:
                    load_p(g + 2)
            if k == 6:
                load_y()
            for g_ in range(len(AG) - 1):
                if k == 8 + 2 * g_:
                    load_a(g_)
            if k == 10:
                emit_smalls()
            if k == 14:
                emit_masks()
            if k == KTP + 2:
                t = dpool.tile((128, 12 * KP), dtb, tag="wos", name="wos")
                nc.sync.dma_start(t, wod)
                sm["wos"] = t
            if k == KTP + 6:
                emit_wom()
            for g in ct_load_step.get(k, []):
                load_c(g)

            # --- q (x) wt opens the ps1 accumulation group ---
            if k == 0:
                for m in range(4):
                    nc.tensor.matmul(
                        pslot(ps1, m), sm["rs"][0:1, m * 128:(m + 1) * 128],
                        sm["rs"][0:1, NS:NS + KP],
                        start=(m % 2 == 0), stop=False)

            # --- G1 matmuls (4 m-blocks into 2 shared banks) ---
            if is_p:
                g = pg_of[kk]
                rhs = wx8_rhs(kk)
                lt = ptg[g]
                jo = kk - PG[g]
            else:
                g = ag_of[kk]
                rhs = dyg[0][:, kk * KP:(kk + 1) * KP]
                lt = atg[g]
                jo = kk - AG[g]

            def g1a_lhs(kk2, m):
                g2_ = ag_of[kk2]
                jo2 = kk2 - AG[g2_]
                return atg[g2_][:, jo2 * NS + m * 128:jo2 * NS + (m + 1) * 128]

            if is_p or kta < 6 or kk < kta - 3:
                for m in range(4):
                    nc.tensor.matmul(
                        pslot(ps1, m),
                        lt[:, jo * NS + m * 128:jo * NS + (m + 1) * 128],
                        rhs, start=False,
                        stop=(not is_p and kta < 6 and kk == kta - 1
                              and m % 2 == 1))
            elif kk == kta - 3:
                # staggered tail: finish bank0, evict+write, then bank1
                for bank, ms in ((0, (0, 1)), (1, (2, 3))):
                    for m in ms:
                        for kk2 in range(kta - 3, kta):
                            nc.tensor.matmul(
                                pslot(ps1, m), g1a_lhs(kk2, m),
                                dyg[0][:, kk2 * KP:(kk2 + 1) * KP],
                                start=False,
                                stop=(m == ms[1] and kk2 == kta - 1))
                    lo = 2 * bank * KP
                    nc.vector.tensor_scalar_mul(
                        ob1[:, lo:lo + KP], pslot(ps1, 2 * bank), c_inv)
                    nc.scalar.activation(
                        ob1[:, lo + KP:lo + 2 * KP], pslot(ps1, 2 * bank + 1),
                        AFT.Copy, scale=c_inv)
                    oeng = nc.sync if bank == 0 else nc.scalar
                    oeng.dma_start(out1[:, lo:lo + 2 * KP],
                                   ob1[:, lo:lo + 2 * KP])

            # --- b (x) wt opens the ps2 accumulation group ---
            if k == 1:
                for t_i in range(8):
                    nc.tensor.matmul(
                        pslot(ps2, t_i),
                        sm["rs"][0:1, NS + KP + t_i * 128:NS + KP + (t_i + 1) * 128],
                        sm["rs"][0:1, NS:NS + KP],
                        start=(t_i % 2 == 0), stop=False)

            # --- G2 ticks ---
            for t in t2s.get(k, []):
                if g2_dr:
                    pair = t
                    g = cg_of[pair]
                    po = pair - CG[g]
                    for t_i in range(8):
                        nc.tensor.matmul(
                            pslot(ps2, t_i),
                            ctg[g][:, po, t_i * 256:(t_i + 1) * 256],
                            wx8_rhs_pair(pair),
                            start=False,
                            stop=(pair == KT2 // 2 - 1 and t_i % 2 == 1),
                            perf_mode=pm)
                else:
                    j = t
                    g = cg_of[j]
                    jo = j - CG[g]
                    for t_i in range(8):
                        nc.tensor.matmul(
                            pslot(ps2, t_i),
                            ctg[g][:, jo, t_i * 128:(t_i + 1) * 128],
                            wx8_rhs(j),
                            start=False,
                            stop=(j == KT2 - 1 and t_i % 2 == 1))
                done_ticks += 1

            # --- o2 eviction + o3 once G2 is done ---
            if done_ticks == nticks:
                done_ticks = -1
                for t_i in range(8):
                    sl = slice(t_i * KP, (t_i + 1) * KP)
                    nc.vector.scalar_tensor_tensor(
                        ob2[:, sl], pslot(ps2, t_i), c_inv, sm["wom"][:, sl],
                        op.mult, op.add)
                nc.scalar.dma_start(out2, ob2)
                pso3 = pspool.tile((1, KP), dtf, tag="ps", name="pso3")
                for t_i in range(8):
                    nc.tensor.matmul(pso3, sm["cnb"][:, 4 + t_i:5 + t_i],
                                     sm["wmt"][:, t_i * KP:(t_i + 1) * KP],
                                     start=(t_i == 0), stop=False)
                for j3 in range(4):
                    nc.tensor.matmul(pso3, sm["cnb"][:, j3:j3 + 1],
                                     sm["wos"][:, (8 + j3) * KP:(9 + j3) * KP],
                                     start=False, stop=(j3 == 3))
                o3f = opool.tile((1, KP), dtf, tag="o3f", name="o3f")
                nc.vector.tensor_copy(o3f, pso3)
                nc.sync.dma_start(out3, o3f)

        # --- final o1 eviction (only if the staggered tail didn't run) ---
        if kta < 6:
            nc.vector.tensor_scalar_mul(ob1[:, 0:KP], pslot(ps1, 0), c_inv)
            nc.scalar.activation(ob1[:, 2 * KP:3 * KP], pslot(ps1, 2),
                                 AFT.Copy, scale=c_inv)
            nc.vector.tensor_scalar_mul(ob1[:, KP:2 * KP], pslot(ps1, 1), c_inv)
            nc.scalar.activation(ob1[:, 3 * KP:4 * KP], pslot(ps1, 3),
                                 AFT.Copy, scale=c_inv)
            nc.sync.dma_start(out1, ob1)

    nc.compile()
    return nc


def _get_nc(key):
    if key not in _NC_CACHE:
        _NC_CACHE[key] = _build_nc(*key)
    return _NC_CACHE[key]


def _pow2_scale(std, mx, limit):
    if not np.isfinite(std) or std <= 0:
        return 1.0
    s = 2.0 ** round(np.log2(1.0 / std))
    while mx * s > limit:
        s *= 0.5
    return s


def _prep(P, A, q, b, x, y, s, W):
    P = np.asarray(P, np.float32)
    A = np.asarray(A, np.float32)
    q = np.asarray(q, np.float32)
    b = np.asarray(b, np.float32)
    x = np.asarray(x, np.float32)
    y = np.asarray(y, np.float32)
    s = np.asarray(s, np.float32)
    W = np.asarray(W, np.float32)

    mb = (y - s) >= 0
    idx = np.nonzero(mb)[0]
    mp = max(1, len(idx))
    kta = (mp + 127) // 128

    wx, wy, wt = W[:N], W[N:N + M], W[N + M:]
    SA = _pow2_scale(A.std(), np.abs(A).max(), 14.0)
    SW = _pow2_scale(1.0, np.abs(W).max(), 14.0)
    c_inv = 1.0 / (SA * SW)

    Px = P @ x
    xPx = float(x @ Px)
    cf = -(q + 2.0 * Px)

    drsw = G2_MODE == "drsw"
    E4c = E4 if drsw else E3
    wx8_h = _kt((wx * SW).astype(E4c), KT2, KP).reshape(128, KT2, KP)
    at_q = (A[idx] * SA).astype(E3)          # (mp, N), quantize once
    dy_full = np.zeros((kta * 128, KP), E3)
    dy_full[:mp] = (wy[idx] * SW).astype(E3)
    dy_h = _kt(dy_full, kta, KP)

    in_maps = []
    for i in range(NC):
        ncol = slice(i * NS, (i + 1) * NS)
        mrow = slice(i * MS, (i + 1) * MS)
        pt0 = (P[:, ncol] * SA).astype(E3)                   # (N, NS)
        at0 = np.zeros((kta * 128, NS), E3)
        at0[:mp] = at_q[:, ncol]
        ct0 = (-(SA * A[mrow].T)).astype(E4c)                # (N, MS)
        if drsw:
            # SW-interleave: flat[p, pr, t, 2*(127-m)+i] = ktile(2pr+i)[p, t, m]
            X = _kt(ct0, KT2, MS).reshape(128, KT2, 8, 128)
            ct_h = np.ascontiguousarray(
                X.reshape(128, KT2 // 2, 2, 8, 128)
                .transpose(0, 1, 3, 4, 2)[:, :, :, ::-1, :]
                .reshape(128, KT2 // 2, 8 * 2 * 128))
        else:
            ct_h = _kt(ct0, KT2, MS).reshape(128, KT2, MS)
        rs = np.concatenate([q[ncol] * SA * SW, wt[0], b[mrow] * SA * SW])
        cs = np.concatenate([cf[ncol].reshape(4, 128).T,
                             (-b[mrow]).reshape(8, 128).T,
                             y[mrow].reshape(8, 128).T,
                             s[mrow].reshape(8, 128).T], axis=1)
        in_maps.append(dict(
            pt8=_kt(pt0, KTP, NS),
            at8=_kt(at0, kta, NS), dy8=dy_h,
            ct8=ct_h, wx8=wx8_h,
            rs=np.ascontiguousarray(rs[None, :].astype(BF)),
            cs=np.ascontiguousarray(cs.astype(np.float32)),
            wosb=_kt(np.vstack([wy[mrow], wx[ncol]]).astype(BF), 12, KP),
        ))
    return in_maps, kta, c_inv, xPx, wt


def _assemble(results, xPx, wt):
    Fo = np.empty((N + M + 1, KP), np.float32)
    o3 = xPx * wt[0].astype(np.float32)
    for i in range(NC):
        o1 = np.asarray(results[i]["out1"], np.float32)     # (128, 4*KP)
        o2 = np.asarray(results[i]["out2"], np.float32)     # (128, 8*KP)
        Fo[i * NS:(i + 1) * NS] = (
            o1.reshape(128, 4, KP).transpose(1, 0, 2).reshape(NS, KP))
        Fo[N + i * MS:N + (i + 1) * MS] = (
            o2.reshape(128, 8, KP).transpose(1, 0, 2).reshape(MS, KP))
        o3 = o3 + np.asarray(results[i]["out3"], np.float32)[0]
    Fo[N + M] = o3
    return Fo


def _run_sharded(inputs, trace=False, trace_kwargs=None):
    from concourse import bass_utils
    in_maps, kta, c_inv, xPx, wt = _prep(**inputs)
    nc = _get_nc((kta, G2_MODE == "drsw", c_inv))
    res = bass_utils.run_bass_kernel_spmd(
        nc, in_maps, core_ids=list(range(NC)), trace=trace,
        **(trace_kwargs or {}))
    return _assemble(res.results, xPx, wt), res


def kernel(**inputs) -> np.ndarray:
    out, _ = _run_sharded(inputs, trace=False)
    return out


# revision 74
# speedup vs baseline: 1.0885x; 1.0885x over previous
"""Trainium2 Bass kernel for the AbstractQCP residual operator F @ W.

Math (reference):
    v = y - s; mask = (v >= 0)
    dx = wx; dy = mask*wy; dt = wt        (W = [wx; wy; wt], (n+m+1, K))
    o1 = P@wx + A.T@dy + q wt             (n, K)
    o2 = b wt - A@wx                      (m, K)
    o3 = (x.T P x) wt - (q + 2 P x)@wx - b@dy
    F  = [o1; o2 + (1-mask)*wy; o3]

Design (per core i of 8, pure SPMD, host gathers):
  core i owns o1 rows [512i,512(i+1)) and o2 rows [1024i,1024(i+1)).
  Host precomputes: mask, row-compacted A.T@dy operands (only rows with
  mask=1 contribute), Px = P@x, xTPx, cf = -(q+2Px).
  All big operands fp8 E3M4 scaled by a single power-of-two S (=64):
    G1P: lhsT = S*P[:,cols_i] (32 ktiles; P symmetric), rhs = wx8 = wx
    G1A: lhsT = S*A[maskrows, cols_i] (compacted ktiles), rhs = dy8
    q x) wt: contraction-1 bf16 matmul, lhsT = S*q_i, rhs = wt row
    --> all accumulate in ONE psum set (identical scale); o1 = ps1/S.
    G2:  lhsT = -S*A[rows_i,:].T (32 ktiles), rhs = wx8 (shared tiles!);
         b wt via contraction-1 bf16 matmul lhsT = S*b_i.
         o2 = ps2/S + (1-mask)*wy.  Optional e4m3+DoubleRow mode.
  o3 partial per core: cf@wx_i + (-b_i)@(mask*wy_i); host adds xTPx*wt.
  PSUM: 2 banks o1 + 4 banks G2 (2 x 256-wide accumulators per bank,
  bank-shared start/stop flags) + 1 bank o3.
  DMA: ~11.6 MB/core balanced over the 3 trigger queues (sync/scalar/
  gpsimd), ~0.5-1 MB per transfer.

Streamed operands staged in DRAM K-tile-transposed: (128, ktiles*free)
with element (p, k*free+c) = orig(k*128+p, c).
"""

import numpy as np
import ml_dtypes
from contextlib import ExitStack

BF = ml_dtypes.bfloat16
E3 = ml_dtypes.float8_e3m4
E4 = ml_dtypes.float8_e4m3

N, M, KP = 4096, 8192, 256
NC = 8
NS, MS = N // NC, M // NC          # 512, 1024
KTP = 32                           # P k-tiles
KT2 = 32                           # G2 k-tiles (full n contraction)

G2_MODE = "drsw"                   # 'e3' | 'drsw' (e4m3 + DoubleRowSwInterleave)

_NC_CACHE = {}


def _kt(a, ktiles, free):
    """(ktiles*128, free) row-major -> (128, ktiles*free) K-tile-transposed."""
    return np.ascontiguousarray(
        a.reshape(ktiles, 128, free).transpose(1, 0, 2).reshape(128, ktiles * free))


def _build_nc(kta, g2_dr, c_inv):
    from concourse import bacc, tile, mybir
    from concourse.alu_op_type import AluOpType as op

    dtb = mybir.dt.bfloat16
    dtf = mybir.dt.float32
    dt8 = mybir.dt.float8e3
    dt8c = mybir.dt.float8e4 if g2_dr else mybir.dt.float8e3
    pm = mybir.MatmulPerfMode.DoubleRowSwInterleave if g2_dr else None

    nc = bacc.Bacc("TRN2", target_bir_lowering=False, debug=False)

    def din(name, shape, dt):
        return nc.dram_tensor(name, list(shape), dt, kind="ExternalInput").ap()

    pt8 = din("pt8", (128, KTP * NS), dt8)    # S*P[:,cols] K-tiled
    at8 = din("at8", (128, kta * NS), dt8)    # compacted S*A rows, K-tiled
    dy8 = din("dy8", (128, kta * KP), dt8)    # compacted wy, K-tiled
    if g2_dr:
        # SW-interleaved pairs: (pair, mtile, [2*(127-m)+i]) per partition
        ct8 = din("ct8", (128, KT2 // 2, 8 * 2 * 128), dt8c)
    else:
        ct8 = din("ct8", (128, KT2, MS), dt8c)  # -S*A[rows].T K-tiled
    wx8 = din("wx8", (128, KT2, KP), dt8c)    # wx K-tiled (G1P + G2 rhs)
    # row smalls: [S*q_i | wt | S*b_i] in one row tensor
    rsd = din("rs", (1, NS + KP + MS), dtb)
    # col smalls: [cf(4) | -b(8) | y(8) | s(8)] as fp32 columns
    csd = din("cs", (128, 28), dtf)
    wod = din("wosb", (128, 12 * KP), dtb)    # [own wy (8) | own wx (4)] K-tiled
    out1 = nc.dram_tensor("out1", [128, 4 * KP], dtb, kind="ExternalOutput").ap()
    out2 = nc.dram_tensor("out2", [128, 8 * KP], dtb, kind="ExternalOutput").ap()
    out3 = nc.dram_tensor("out3", [1, KP], dtf, kind="ExternalOutput").ap()

    NSTEP = KTP + kta
    PG = [0, 8, 16, 24, 32]                   # pt8 groups (alternate sync/scalar)
    AG = sorted(set(min(b, kta) for b in [0, 8, 16, 24, kta]))  # at8 (alt)
    # ct8 groups (gpsimd): in pair units for drsw, ktile units for e3
    CG = [0, 4, 8, 12, 16] if g2_dr else [0, 8, 16, 24, 32]
    WXG = [0, 8, 32]                          # wx8 chunks (scalar)

    def g_of(bounds):
        m = {}
        for g in range(len(bounds) - 1):
            for k in range(bounds[g], bounds[g + 1]):
                m[k] = g
        return m

    pg_of, ag_of, cg_of = g_of(PG), g_of(AG), g_of(CG)

    nticks = KT2 // 2 if g2_dr else KT2
    first_tick, last_tick = 14, NSTEP - 10
    tick_step = [first_tick + round(t * (last_tick - first_tick) / (nticks - 1))
                 for t in range(nticks)]
    t2s = {}
    for t, s_ in enumerate(tick_step):
        t2s.setdefault(s_, []).append(t)
    ct_load_step = {}
    for g in range(len(CG) - 1):
        ct_load_step.setdefault(max(3, tick_step[min(CG[g], nticks - 1)] - 6),
                                []).append(g)

    with tile.TileContext(nc) as tc, ExitStack() as ctx:
        dpool = ctx.enter_context(tc.tile_pool(name="d", bufs=1))
        ppool = ctx.enter_context(tc.tile_pool(name="p", bufs=4))
        apool = ctx.enter_context(tc.tile_pool(name="a", bufs=4))
        ypool = ctx.enter_context(tc.tile_pool(name="y", bufs=2))
        cpool = ctx.enter_context(tc.tile_pool(name="c", bufs=2))
        opool = ctx.enter_context(tc.tile_pool(name="o", bufs=1))
        pspool = ctx.enter_context(tc.tile_pool(name="ps", bufs=8, space="PSUM"))

        ps1 = [pspool.tile((128, 2 * KP), dtf, tag="ps", name=f"ps1{i}") for i in range(2)]
        ps2 = [pspool.tile((128, 2 * KP), dtf, tag="ps", name=f"ps2{i}") for i in range(4)]

        def pslot(tiles, t):
            return tiles[t // 2][:, (t % 2) * KP:(t % 2 + 1) * KP]

        ptg, atg, dyg, ctg = {}, {}, {}, {}

        def load_p(g):
            k0, k1 = PG[g], PG[g + 1]
            t = ppool.tile((128, (k1 - k0) * NS), dt8, tag="pt", name=f"ptg{g}",
                           padded_shape=(128, 10 * NS))
            eng = nc.sync if g % 2 == 0 else nc.scalar
            eng.dma_start(t, pt8[:, k0 * NS:k1 * NS])
            ptg[g] = t

        def load_a(g):
            k0, k1 = AG[g], AG[g + 1]
            t = apool.tile((128, (k1 - k0) * NS), dt8, tag="at",
                           name=f"atg{g}", padded_shape=(128, 9 * NS))
            eng = nc.sync if g % 2 == 0 else nc.scalar
            eng.dma_start(t, at8[:, k0 * NS:k1 * NS])
            atg[g] = t

        def load_y():
            t = ypool.tile((128, kta * KP), dt8, tag="dy", name="dyg")
            nc.scalar.dma_start(t, dy8)
            dyg[0] = t

        def load_c(g):
            j0, j1 = CG[g], CG[g + 1]
            gmax = max(b - a for a, b in zip(CG[:-1], CG[1:]))
            if g2_dr:
                t = cpool.tile((128, gmax, 8 * 2 * 128), dt8c, tag="ct", name=f"ctg{g}")
            else:
                t = cpool.tile((128, gmax, MS), dt8c, tag="ct", name=f"ctg{g}")
            nc.gpsimd.dma_start(t[:, 0:j1 - j0, :], ct8[:, j0:j1, :])
            ctg[g] = t

        # wx8 resident: ONE fat-row transfer on gpsimd (SWDGE), ahead of ct8
        wxt = dpool.tile((128, KT2, KP), dt8c, tag="wxt", name="wxt")

        def wx8_rhs(j):
            return wxt[:, j, :]

        def wx8_rhs_pair(p_):
            return wxt[:, 2 * p_:2 * p_ + 2, 0:KP]

        sm = {}

        def emit_first_smalls():
            rs = dpool.tile((1, NS + KP + MS), dtb, tag="rs", name="rs")
            nc.scalar.dma_start(rs, rsd)
            sm["rs"] = rs

        def emit_smalls():
            cs = dpool.tile((128, 28), dtf, tag="cs", name="cs")
            nc.scalar.dma_start(cs, csd)
            sm["cs"] = cs

        def emit_masks():
            cs = sm["cs"]
            vo = dpool.tile((128, 8), dtf, tag="vo", name="vo")
            nc.vector.tensor_sub(vo, cs[:, 12:20], cs[:, 20:28])
            masko = dpool.tile((128, 8), dtf, tag="masko", name="masko")
            nc.vector.tensor_scalar(masko, vo, 0.0, None, op.is_ge)
            umo = dpool.tile((128, 8), dtf, tag="umo", name="umo")
            nc.vector.tensor_scalar(umo, masko, -1.0, 1.0, op.mult, op.add)
            sm["umo"] = umo
            cnb = dpool.tile((128, 12), dtb, tag="cnb", name="cnb")
            nc.vector.tensor_copy(cnb, cs[:, 0:12])
            sm["cnb"] = cnb

        def emit_wom():
            wom = dpool.tile((128, 8 * KP), dtb, tag="wom", name="wom")
            wmt = dpool.tile((128, 8 * KP), dtb, tag="wmt", name="wmt")
            for t_i in range(8):
                sl = slice(t_i * KP, (t_i + 1) * KP)
                nc.vector.tensor_scalar_mul(wom[:, sl], sm["wos"][:, sl],
                                            sm["umo"][:, t_i:t_i + 1])
            for t_i in range(8):
                sl = slice(t_i * KP, (t_i + 1) * KP)
                nc.vector.tensor_sub(wmt[:, sl], sm["wos"][:, sl], wom[:, sl])
            sm["wom"] = wom
            sm["wmt"] = wmt

        from bass_rust import ActivationFunctionType as AFT

        ob1 = opool.tile((128, 4 * KP), dtb, tag="ob1", name="ob1")
        ob2 = opool.tile((128, 8 * KP), dtb, tag="ob2", name="ob2")

        # front-loaded triggers: smalls + wx8 c0/c1 on scalar, pt g0/g1 on sync
        emit_first_smalls()
        nc.gpsimd.dma_start(wxt, wx8)
        load_p(0)
        load_p(1)

        done_ticks = 0
        for k in range(NSTEP):
            is_p = k < KTP
            kk = k if is_p else k - KTP

            # --- JIT stream prefetch ---
            if is_p:
                g = pg_of[kk]
                if kk == PG[g] and g + 2 <= len(PG) - 2:
                    load_p(g + 2)
            if k == 6:
                load_y()
            for g_ in range(len(AG) - 1):
                if k == 8 + 2 * g_:
                    load_a(g_)
            if k == 10:
                emit_smalls()
            if k == 14:
                emit_masks()
            if k == KTP + 2:
                t = dpool.tile((128, 12 * KP), dtb, tag="wos", name="wos")
                nc.sync.dma_start(t, wod)
                sm["wos"] = t
            if k == KTP + 6:
                emit_wom()
            for g in ct_load_step.get(k, []):
                load_c(g)

            # --- q (x) wt opens the ps1 accumulation group ---
            if k == 0:
                for m in range(4):
                    nc.tensor.matmul(
                        pslot(ps1, m), sm["rs"][0:1, m * 128:(m + 1) * 128],
                        sm["rs"][0:1, NS:NS + KP],
                        start=(m % 2 == 0), stop=False)

            # --- G1 matmuls (4 m-blocks into 2 shared banks) ---
            if is_p:
                g = pg_of[kk]
                rhs = wx8_rhs(kk)
                lt = ptg[g]
                jo = kk - PG[g]
            else:
                g = ag_of[kk]
                rhs = dyg[0][:, kk * KP:(kk + 1) * KP]
                lt = atg[g]
                jo = kk - AG[g]

            def g1a_lhs(kk2, m):
                g2_ = ag_of[kk2]
                jo2 = kk2 - AG[g2_]
                return atg[g2_][:, jo2 * NS + m * 128:jo2 * NS + (m + 1) * 128]

            if is_p or kta < 6 or kk < kta - 3:
                for m in range(4):
                    nc.tensor.matmul(
                        pslot(ps1, m),
                        lt[:, jo * NS + m * 128:jo * NS + (m + 1) * 128],
                        rhs, start=False,
                        stop=(not is_p and kta < 6 and kk == kta - 1
                              and m % 2 == 1))
            elif kk == kta - 3:
                # staggered tail: finish bank0, evict+write, then bank1
                for bank, ms in ((0, (0, 1)), (1, (2, 3))):
                    for m in ms:
                        for kk2 in range(kta - 3, kta):
                            nc.tensor.matmul(
                                pslot(ps1, m), g1a_lhs(kk2, m),
                                dyg[0][:, kk2 * KP:(kk2 + 1) * KP],
                                start=False,
                                stop=(m == ms[1] and kk2 == kta - 1))
                    lo = 2 * bank * KP
                    nc.vector.tensor_scalar_mul(
                        ob1[:, lo:lo + KP], pslot(ps1, 2 * bank), c_inv)
                    nc.scalar.activation(
                        ob1[:, lo + KP:lo + 2 * KP], pslot(ps1, 2 * bank + 1),
                        AFT.Copy, scale=c_inv)
                    nc.sync.dma_start(out1[:, lo:lo + 2 * KP],
                                      ob1[:, lo:lo + 2 * KP])

            # --- b (x) wt opens the ps2 accumulation group ---
            if k == 1:
                for t_i in range(8):
                    nc.tensor.matmul(
                        pslot(ps2, t_i),
                        sm["rs"][0:1, NS + KP + t_i * 128:NS + KP + (t_i + 1) * 128],
                        sm["rs"][0:1, NS:NS + KP],
                        start=(t_i % 2 == 0), stop=False)

            # --- G2 ticks ---
            for t in t2s.get(k, []):
                if g2_dr:
                    pair = t
                    g = cg_of[pair]
                    po = pair - CG[g]
                    for t_i in range(8):
                        nc.tensor.matmul(
                            pslot(ps2, t_i),
                            ctg[g][:, po, t_i * 256:(t_i + 1) * 256],
                            wx8_rhs_pair(pair),
                            start=False,
                            stop=(pair == KT2 // 2 - 1 and t_i % 2 == 1),
                            perf_mode=pm)
                else:
                    j = t
                    g = cg_of[j]
                    jo = j - CG[g]
                    for t_i in range(8):
                        nc.tensor.matmul(
                            pslot(ps2, t_i),
                            ctg[g][:, jo, t_i * 128:(t_i + 1) * 128],
                            wx8_rhs(j),
                            start=False,
                            stop=(j == KT2 - 1 and t_i % 2 == 1))
                done_ticks += 1

            # --- o2 eviction + o3 once G2 is done ---
            if done_ticks == nticks:
                done_ticks = -1
                for t_i in range(8):
                    sl = slice(t_i * KP, (t_i + 1) * KP)
                    nc.vector.scalar_tensor_tensor(
                        ob2[:, sl], pslot(ps2, t_i), c_inv, sm["wom"][:, sl],
                        op.mult, op.add)
                nc.scalar.dma_start(out2, ob2)
                pso3 = pspool.tile((1, KP), dtf, tag="ps", name="pso3")
                for t_i in range(8):
                    nc.tensor.matmul(pso3, sm["cnb"][:, 4 + t_i:5 + t_i],
                                     sm["wmt"][:, t_i * KP:(t_i + 1) * KP],
                                     start=(t_i == 0), stop=False)
                for j3 in range(4):
                    nc.tensor.matmul(pso3, sm["cnb"][:, j3:j3 + 1],
                                     sm["wos"][:, (8 + j3) * KP:(9 + j3) * KP],
                                     start=False, stop=(j3 == 3))
                o3f = opool.tile((1, KP), dtf, tag="o3f", name="o3f")
                nc.vector.tensor_copy(o3f, pso3)
                nc.sync.dma_start(out3, o3f)

        # --- final o1 eviction (only if the staggered tail didn't run) ---
        if kta < 6:
            nc.vector.tensor_scalar_mul(ob1[:, 0:KP], pslot(ps1, 0), c_inv)
            nc.scalar.activation(ob1[:, 2 * KP:3 * KP], pslot(ps1, 2),
                                 AFT.Copy, scale=c_inv)
            nc.vector.tensor_scalar_mul(ob1[:, KP:2 * KP], pslot(ps1, 1), c_inv)
            nc.scalar.activation(ob1[:, 3 * KP:4 * KP], pslot(ps1, 3),
                                 AFT.Copy, scale=c_inv)
            nc.sync.dma_start(out1, ob1)

    nc.compile()
    return nc


def _get_nc(key):
    if key not in _NC_CACHE:
        _NC_CACHE[key] = _build_nc(*key)
    return _NC_CACHE[key]


def _pow2_scale(std, mx, limit):
    if not np.isfinite(std) or std <= 0:
        return 1.0
    s = 2.0 ** round(np.log2(1.0 / std))
    while mx * s > limit:
        s *= 0.5
    return s


def _prep(P, A, q, b, x, y, s, W):
    P = np.asarray(P, np.float32)
    A = np.asarray(A, np.float32)
    q = np.asarray(q, np.float32)
    b = np.asarray(b, np.float32)
    x = np.asarray(x, np.float32)
    y = np.asarray(y, np.float32)
    s = np.asarray(s, np.float32)
    W = np.asarray(W, np.float32)

    mb = (y - s) >= 0
    idx = np.nonzero(mb)[0]
    mp = max(1, len(idx))
    kta = (mp + 127) // 128

    wx, wy, wt = W[:N], W[N:N + M], W[N + M:]
    SA = _pow2_scale(A.std(), np.abs(A).max(), 14.0)
    SW = _pow2_scale(1.0, np.abs(W).max(), 14.0)
    c_inv = 1.0 / (SA * SW)

    Px = P @ x
    xPx = float(x @ Px)
    cf = -(q + 2.0 * Px)

    drsw = G2_MODE == "drsw"
    E4c = E4 if drsw else E3
    wx8_h = _kt((wx * SW).astype(E4c), KT2, KP).reshape(128, KT2, KP)
    at_q = (A[idx] * SA).astype(E3)          # (mp, N), quantize once
    dy_full = np.zeros((kta * 128, KP), E3)
    dy_full[:mp] = (wy[idx] * SW).astype(E3)
    dy_h = _kt(dy_full, kta, KP)

    in_maps = []
    for i in range(NC):
        ncol = slice(i * NS, (i + 1) * NS)
        mrow = slice(i * MS, (i + 1) * MS)
        pt0 = (P[:, ncol] * SA).astype(E3)                   # (N, NS)
        at0 = np.zeros((kta * 128, NS), E3)
        at0[:mp] = at_q[:, ncol]
        ct0 = (-(SA * A[mrow].T)).astype(E4c)                # (N, MS)
        if drsw:
            # SW-interleave: flat[p, pr, t, 2*(127-m)+i] = ktile(2pr+i)[p, t, m]
            X = _kt(ct0, KT2, MS).reshape(128, KT2, 8, 128)
            ct_h = np.ascontiguousarray(
                X.reshape(128, KT2 // 2, 2, 8, 128)
                .transpose(0, 1, 3, 4, 2)[:, :, :, ::-1, :]
                .reshape(128, KT2 // 2, 8 * 2 * 128))
        else:
            ct_h = _kt(ct0, KT2, MS).reshape(128, KT2, MS)
        rs = np.concatenate([q[ncol] * SA * SW, wt[0], b[mrow] * SA * SW])
        cs = np.concatenate([cf[ncol].reshape(4, 128).T,
                             (-b[mrow]).reshape(8, 128).T,
                             y[mrow].reshape(8, 128).T,
                             s[mrow].reshape(8, 128).T], axis=1)
        in_maps.append(dict(
            pt8=_kt(pt0, KTP, NS),
            at8=_kt(at0, kta, NS), dy8=dy_h,
            ct8=ct_h, wx8=wx8_h,
            rs=np.ascontiguousarray(rs[None, :].astype(BF)),
            cs=np.ascontiguousarray(cs.astype(np.float32)),
            wosb=_kt(np.vstack([wy[mrow], wx[ncol]]).astype(BF), 12, KP),
        ))
    return in_maps, kta, c_inv, xPx, wt


def _assemble(results, xPx, wt):
    Fo = np.empty((N + M + 1, KP), np.float32)
    o3 = xPx * wt[0].astype(np.float32)
    for i in range(NC):
        o1 = np.asarray(results[i]["out1"], np.float32)     # (128, 4*KP)
        o2 = np.asarray(results[i]["out2"], np.float32)     # (128, 8*KP)
        Fo[i * NS:(i + 1) * NS] = (
            o1.reshape(128, 4, KP).transpose(1, 0, 2).reshape(NS, KP))
        Fo[N + i * MS:N + (i + 1) * MS] = (
            o2.reshape(128, 8, KP).transpose(1, 0, 2).reshape(MS, KP))
        o3 = o3 + np.asarray(results[i]["out3"], np.float32)[0]
    Fo[N + M] = o3
    return Fo


def _run_sharded(inputs, trace=False, trace_kwargs=None):
    from concourse import bass_utils
    in_maps, kta, c_inv, xPx, wt = _prep(**inputs)
    nc = _get_nc((kta, G2_MODE == "drsw", c_inv))
    res = bass_utils.run_bass_kernel_spmd(
        nc, in_maps, core_ids=list(range(NC)), trace=trace,
        **(trace_kwargs or {}))
    return _assemble(res.results, xPx, wt), res


def kernel(**inputs) -> np.ndarray:
    out, _ = _run_sharded(inputs, trace=False)
    return out
